# revision 9
# baseline (speedup 1.0000x reference)
"""Cosine-similarity attention map on 8 Trainium2 NeuronCores.

out[b, i, j] = <x[b,:,i], x[b,:,j]> / (||x[b,:,i]|| * ||x[b,:,j]||)
x: [B=4, C=64, N=4096] fp32  ->  out: [B=4, N=4096, N=4096] fp32

Sharding: data-parallel over B (4 batches) x 2-way row-split of the N x N
output -> 8 cores. Each core receives the full x[b] (for the moving operand
and column norms) plus its 2048-column row slice (for the stationary
operand), normalizes columns on device (y = x * rsqrt(sum_c x^2)), and
computes its [2048, 4096] block of the Gram matrix of y with fp32r matmuls.
"""

import sys

sys.path.insert(0, "/opt/trn_rl_repo")

import numpy as np

import concourse.bass as bass
import concourse.mybir as mybir
import concourse.tile as tile
from concourse import bacc
from concourse.bass_utils import run_bass_kernel_spmd
from concourse.vector_clock import ScopedClock, VectorClock

B, C, N = 4, 64, 4096
NCORES = 8
RB = N * B // NCORES  # 2048 output rows per core
MM_N = 512  # moving free dim per matmul (one PSUM bank of fp32)
MM_M = 128  # output partitions per matmul
NJ = N // MM_N  # 8 column chunks
NT = RB // MM_M  # 16 row tiles per core

F32 = mybir.dt.float32
F32R = mybir.dt.float32r


class SplitDrainTileContext(tile.TileContext):
    """Stock TileContext attaches a wait for every pending DMA-queue
    semaphore to a single exit Drain; the walrus build here only allows one
    sync-wait per TPB_CTRL instruction ("Too many sync wait commands").
    Emit one drain per pending logical processor instead."""

    def _drain_and_barrier(self, tick_clock, wait_clock):
        gc = tick_clock.global_clock
        n = len(gc)
        for p in range(n):
            t = gc[p]
            if t <= 0:
                continue
            part = VectorClock([t if q == p else 0 for q in range(n)])
            d = self.nc.sync.drain()
            wait_clock.add_sem_waits(d.ins, ScopedClock({None: part}))

        self.nc.all_engine_barrier()
        assert self.sems is not None
        popped = self.nc._tile_sem_poison_stack.pop()
        assert popped is self._sem_poison
        self.nc.clear_and_free_semaphores(list(self.sems.allocated().values()))
        self.nc.all_engine_barrier()


def _build(use_split_drain=False):
    nc = bacc.Bacc("TRN2", target_bir_lowering=False)
    xf = nc.declare_dram_parameter("xf", [C, N], F32, isOutput=False)
    xr = nc.declare_dram_parameter("xr", [C, RB], F32, isOutput=False)
    out = nc.declare_dram_parameter("out", [RB, N], F32, isOutput=True)
    rnf_d = nc.dram_tensor("rnf_bounce", [1, N], F32)
    rnr_d = nc.dram_tensor("rnr_bounce", [1, RB], F32)

    tc_cls = SplitDrainTileContext if use_split_drain else tile.TileContext
    with tc_cls(nc) as tc:
        with (
            tc.tile_pool(name="persist", bufs=1) as persist,
            tc.tile_pool(name="panels", bufs=3) as panels,
            tc.tile_pool(name="mpsum", bufs=6, space="PSUM") as mpsum,
            tc.tile_pool(name="npsum", bufs=2, space="PSUM") as npsum,
        ):
            # Load inputs.
            XF = persist.tile([C, N], F32)
            XR = persist.tile([C, RB], F32)
            nc.sync.dma_start(out=XF, in_=xf[:, :])
            nc.sync.dma_start(out=XR, in_=xr[:, :])

            # Column sum of squares via matmul with a ones vector
            # (reduction over the partition axis), then rsqrt.
            SQF = persist.tile([C, N], F32R)
            SQR = persist.tile([C, RB], F32R)
            nc.vector.tensor_mul(SQF, XF, XF)
            nc.vector.tensor_mul(SQR, XR, XR)

            ones_f = persist.tile([C, 1], F32)
            nc.vector.memset(ones_f, 1.0)
            ones = persist.tile([C, 1], F32R)
            nc.vector.tensor_copy(ones, ones_f)

            RNF = persist.tile([1, N], F32)  # 1/||x[:, i]|| for all columns
            RNR = persist.tile([1, RB], F32)  # same for this core's rows
            for j in range(NJ):
                js = slice(j * MM_N, (j + 1) * MM_N)
                ps = npsum.tile([1, MM_N], F32)
                nc.tensor.matmul(
                    ps,
                    lhsT=ones,
                    rhs=SQF[:, js],
                    start=True,
                    stop=True,
                )
                nc.scalar.activation(
                    RNF[:, js], ps, mybir.ActivationFunctionType.Sqrt
                )
            nc.vector.reciprocal(RNF, RNF)
            for j in range(RB // MM_N):
                js = slice(j * MM_N, (j + 1) * MM_N)
                ps = npsum.tile([1, MM_N], F32)
                nc.tensor.matmul(
                    ps,
                    lhsT=ones,
                    rhs=SQR[:, js],
                    start=True,
                    stop=True,
                )
                nc.scalar.activation(
                    RNR[:, js], ps, mybir.ActivationFunctionType.Sqrt
                )
            nc.vector.reciprocal(RNR, RNR)

            # Broadcast the [1, n] reciprocal norms across C partitions and
            # scale columns: y = x * rnorm.
            BF = persist.tile([C, N], F32)
            BR = persist.tile([C, RB], F32)
            nc.sync.dma_start(out=rnf_d[:, :], in_=RNF)
            nc.sync.dma_start(out=rnr_d[:, :], in_=RNR)
            nc.sync.dma_start(out=BF, in_=rnf_d[:, :].to_broadcast([C, N]))
            nc.sync.dma_start(out=BR, in_=rnr_d[:, :].to_broadcast([C, RB]))

            YF = persist.tile([C, N], F32R)
            YR = persist.tile([C, RB], F32R)
            nc.vector.tensor_mul(YF, XF, BF)
            nc.vector.tensor_mul(YR, XR, BR)

            # Gram matrix: out[i, j] = sum_c YR[c, i] * YF[c, j].
            # Row-panel at a time so each output DMA is one contiguous 2 MiB.
            for t in range(NT):
                panel = panels.tile([MM_M, N], F32)
                ts_ = slice(t * MM_M, (t + 1) * MM_M)
                for j in range(NJ):
                    js = slice(j * MM_N, (j + 1) * MM_N)
                    ps = mpsum.tile([MM_M, MM_N], F32)
                    nc.tensor.matmul(
                        ps,
                        lhsT=YR[:, ts_],
                        rhs=YF[:, js],
                        start=True,
                        stop=True,
                    )
                    nc.vector.tensor_copy(panel[:, js], ps)
                nc.sync.dma_start(out=out[ts_, :], in_=panel)

    nc.compile()
    return nc


def _install_profile_hook():
    """This container's antenv lacks axon_hooks, so run_bass_kernel_spmd's
    trace=True path dies on import. Recreate the module and register the
    ctypes NTFF hook that trn_boot would have installed."""
    import sys as _sys
    import types

    if "antenv.axon_hooks" in _sys.modules:
        return
    import antenv

    mod = types.ModuleType("antenv.axon_hooks")
    mod._hook = None

    def set_axon_ntff_profile_hook(h):
        mod._hook = h

    def get_axon_ntff_profile_hook():
        return mod._hook

    mod.set_axon_ntff_profile_hook = set_axon_ntff_profile_hook
    mod.get_axon_ntff_profile_hook = get_axon_ntff_profile_hook
    _sys.modules["antenv.axon_hooks"] = mod
    antenv.axon_hooks = mod

    from trn_agent_boot.trn_boot import _ntff_profile_via_ctypes

    mod.set_axon_ntff_profile_hook(
        _ntff_profile_via_ctypes("/opt/axon/libaxon_pjrt.so")
    )


_nc = None


def _get_nc():
    global _nc
    if _nc is None:
        _nc = _build()
    return _nc


def _run(x, trace=False, trace_cores=None):
    x = np.asarray(x, dtype=np.float32)
    assert x.shape == (B, C, N), x.shape
    core_ids = list(range(NCORES))
    in_maps = []
    for k in core_ids:
        b, r = divmod(k, 2)
        in_maps.append(
            {
                "xf": np.ascontiguousarray(x[b]),
                "xr": np.ascontiguousarray(x[b][:, r * RB : (r + 1) * RB]),
            }
        )
    if trace:
        _install_profile_hook()
    res = run_bass_kernel_spmd(
        _get_nc(), in_maps, core_ids, trace=trace, trace_cores=trace_cores
    )
    out = np.empty((B, N, N), dtype=np.float32)
    for k in core_ids:
        b, r = divmod(k, 2)
        out[b, r * RB : (r + 1) * RB, :] = res.results[k]["out"]
    return out, res


def kernel(x):
    return _run(x)[0]


# revision 10
# speedup vs baseline: 1.0622x; 1.0622x over previous
"""Cosine-similarity attention map on 8 Trainium2 NeuronCores.

out[b, i, j] = <x[b,:,i], x[b,:,j]> / (||x[b,:,i]|| * ||x[b,:,j]||)
x: [B=4, C=64, N=4096] fp32  ->  out: [B=4, N=4096, N=4096] fp32

Sharding: data-parallel over B (4 batches) x 2-way row-split of the N x N
output -> 8 cores. Each core receives the full x[b] (for the moving operand
and column norms) plus its 2048-column row slice (for the stationary
operand), normalizes columns on device (y = x * rsqrt(sum_c x^2)), and
computes its [2048, 4096] block of the Gram matrix of y with fp32r matmuls.
"""

import sys

sys.path.insert(0, "/opt/trn_rl_repo")

import numpy as np

import concourse.bass as bass
import concourse.mybir as mybir
import concourse.tile as tile
from concourse import bacc
from concourse.bass_utils import run_bass_kernel_spmd
from concourse.vector_clock import ScopedClock, VectorClock

B, C, N = 4, 64, 4096
NCORES = 8
RB = N * B // NCORES  # 2048 output rows per core
MM_N = 512  # moving free dim per matmul (one PSUM bank of fp32)
MM_M = 128  # output partitions per matmul
NJ = N // MM_N  # 8 column chunks
NT = RB // MM_M  # 16 row tiles per core

F32 = mybir.dt.float32
F32R = mybir.dt.float32r


class SplitDrainTileContext(tile.TileContext):
    """Stock TileContext attaches a wait for every pending DMA-queue
    semaphore to a single exit Drain; the walrus build here only allows one
    sync-wait per TPB_CTRL instruction ("Too many sync wait commands").
    Emit one drain per pending logical processor instead."""

    def _drain_and_barrier(self, tick_clock, wait_clock):
        gc = tick_clock.global_clock
        n = len(gc)
        for p in range(n):
            t = gc[p]
            if t <= 0:
                continue
            part = VectorClock([t if q == p else 0 for q in range(n)])
            d = self.nc.sync.drain()
            wait_clock.add_sem_waits(d.ins, ScopedClock({None: part}))

        self.nc.all_engine_barrier()
        assert self.sems is not None
        popped = self.nc._tile_sem_poison_stack.pop()
        assert popped is self._sem_poison
        self.nc.clear_and_free_semaphores(list(self.sems.allocated().values()))
        self.nc.all_engine_barrier()


def _build(use_split_drain=False):
    nc = bacc.Bacc("TRN2", target_bir_lowering=False)
    xf = nc.declare_dram_parameter("xf", [C, N], F32, isOutput=False)
    xr = nc.declare_dram_parameter("xr", [C, RB], F32, isOutput=False)
    out = nc.declare_dram_parameter("out", [RB, N], F32, isOutput=True)
    rnf_d = nc.dram_tensor("rnf_bounce", [1, N], F32)
    rnr_d = nc.dram_tensor("rnr_bounce", [1, RB], F32)

    tc_cls = SplitDrainTileContext if use_split_drain else tile.TileContext
    with tc_cls(nc) as tc:
        with (
            tc.tile_pool(name="persist", bufs=1) as persist,
            tc.tile_pool(name="panels", bufs=3) as panels,
            tc.tile_pool(name="mpsum", bufs=2, space="PSUM") as mpsum,
        ):
            # Load inputs.
            XF = persist.tile([C, N], F32)
            XR = persist.tile([C, RB], F32)
            nc.sync.dma_start(out=XR, in_=xr[:, :])
            nc.sync.dma_start(out=XF, in_=xf[:, :])

            ones_f = persist.tile([C, 1], F32)
            nc.vector.memset(ones_f, 1.0)
            ones = persist.tile([C, 1], F32R)
            nc.vector.tensor_copy(ones, ones_f)

            # ---- Row path (gates the matmuls): normalize this core's row
            # columns: yr = xr * rsqrt(sum_c xr^2).
            SQR = persist.tile([C, RB], F32R)
            nc.vector.tensor_mul(SQR, XR, XR)
            RNR = persist.tile([1, RB], F32)
            nps = mpsum.tile([MM_M, 4 * MM_N], F32, tag="ps")
            for q in range(RB // MM_N):
                js = slice(q * MM_N, (q + 1) * MM_N)
                nc.tensor.matmul(
                    nps[0:1, js], lhsT=ones, rhs=SQR[:, js], start=True, stop=True
                )
            nc.scalar.activation(
                RNR, nps[0:1, :], mybir.ActivationFunctionType.Sqrt
            )
            nc.vector.reciprocal(RNR, RNR)
            BR = persist.tile([C, RB], F32)
            nc.sync.dma_start(out=rnr_d[:, :], in_=RNR)
            nc.sync.dma_start(out=BR, in_=rnr_d[:, :].to_broadcast([C, RB]))
            YR = persist.tile([C, RB], F32R)
            nc.vector.tensor_mul(YR, XR, BR)

            # fp32r-rounded copy of the full x (moving operand). On the
            # scalar engine to keep DVE free.
            XFr = persist.tile([C, N], F32R)
            nc.scalar.copy(out=XFr, in_=XF)

            # ---- Column path (gates only the PSUM->SBUF stage): reciprocal
            # norms of all N columns, broadcast to 128 partitions.
            SQF = persist.tile([C, N], F32R)
            nc.vector.tensor_mul(SQF, XF, XF)
            RNF = persist.tile([1, N], F32)
            for h in range(2):
                nps = mpsum.tile([MM_M, 4 * MM_N], F32, tag="ps")
                for q in range(4):
                    j = 4 * h + q
                    js = slice(j * MM_N, (j + 1) * MM_N)
                    qs = slice(q * MM_N, (q + 1) * MM_N)
                    nc.tensor.matmul(
                        nps[0:1, qs], lhsT=ones, rhs=SQF[:, js], start=True, stop=True
                    )
                nc.scalar.activation(
                    RNF[:, 2048 * h : 2048 * (h + 1)],
                    nps[0:1, :],
                    mybir.ActivationFunctionType.Sqrt,
                )
            nc.vector.reciprocal(RNF, RNF)
            R128 = persist.tile([MM_M, N], F32)
            nc.sync.dma_start(out=rnf_d[:, :], in_=RNF)
            nc.sync.dma_start(out=R128, in_=rnf_d[:, :].to_broadcast([MM_M, N]))

            # ---- Main loop: out[i, j] = (yr_i . x_j) * rnf_j.
            # 4 matmuls fill a 4-bank PSUM tile; one DVE multiply folds the
            # column normalization while moving PSUM->SBUF; one contiguous
            # 2 MiB DMA per 128-row panel.
            for t in range(NT):
                panel = panels.tile([MM_M, N], F32)
                ts_ = slice(t * MM_M, (t + 1) * MM_M)
                for h in range(2):
                    ps = mpsum.tile([MM_M, 4 * MM_N], F32, tag="ps")
                    for q in range(4):
                        j = 4 * h + q
                        js = slice(j * MM_N, (j + 1) * MM_N)
                        qs = slice(q * MM_N, (q + 1) * MM_N)
                        nc.tensor.matmul(
                            ps[:, qs],
                            lhsT=YR[:, ts_],
                            rhs=XFr[:, js],
                            start=True,
                            stop=True,
                        )
                    hs = slice(h * 2048, (h + 1) * 2048)
                    nc.vector.tensor_mul(panel[:, hs], ps, R128[:, hs])
                nc.sync.dma_start(out=out[ts_, :], in_=panel)

    nc.compile()
    return nc


def _install_profile_hook():
    """This container's antenv lacks axon_hooks, so run_bass_kernel_spmd's
    trace=True path dies on import. Recreate the module and register the
    ctypes NTFF hook that trn_boot would have installed."""
    import sys as _sys
    import types

    if "antenv.axon_hooks" in _sys.modules:
        return
    import antenv

    mod = types.ModuleType("antenv.axon_hooks")
    mod._hook = None

    def set_axon_ntff_profile_hook(h):
        mod._hook = h

    def get_axon_ntff_profile_hook():
        return mod._hook

    mod.set_axon_ntff_profile_hook = set_axon_ntff_profile_hook
    mod.get_axon_ntff_profile_hook = get_axon_ntff_profile_hook
    _sys.modules["antenv.axon_hooks"] = mod
    antenv.axon_hooks = mod

    from trn_agent_boot.trn_boot import _ntff_profile_via_ctypes

    mod.set_axon_ntff_profile_hook(
        _ntff_profile_via_ctypes("/opt/axon/libaxon_pjrt.so")
    )


_nc = None


def _get_nc():
    global _nc
    if _nc is None:
        _nc = _build()
    return _nc


def _run(x, trace=False, trace_cores=None):
    x = np.asarray(x, dtype=np.float32)
    assert x.shape == (B, C, N), x.shape
    core_ids = list(range(NCORES))
    in_maps = []
    for k in core_ids:
        b, r = divmod(k, 2)
        in_maps.append(
            {
                "xf": np.ascontiguousarray(x[b]),
                "xr": np.ascontiguousarray(x[b][:, r * RB : (r + 1) * RB]),
            }
        )
    if trace:
        _install_profile_hook()
    res = run_bass_kernel_spmd(
        _get_nc(), in_maps, core_ids, trace=trace, trace_cores=trace_cores
    )
    out = np.empty((B, N, N), dtype=np.float32)
    for k in core_ids:
        b, r = divmod(k, 2)
        out[b, r * RB : (r + 1) * RB, :] = res.results[k]["out"]
    return out, res


def kernel(x):
    return _run(x)[0]


# revision 11
# speedup vs baseline: 1.1681x; 1.0996x over previous
"""Cosine-similarity attention map on 8 Trainium2 NeuronCores.

out[b, i, j] = <x[b,:,i], x[b,:,j]> / (||x[b,:,i]|| * ||x[b,:,j]||)
x: [B=4, C=64, N=4096] fp32  ->  out: [B=4, N=4096, N=4096] fp32

Sharding: data-parallel over B (4 batches) x 2-way row-split of the N x N
output -> 8 cores. Each core receives the full x[b] (for the moving operand
and column norms) plus its 2048-column row slice (for the stationary
operand), normalizes columns on device (y = x * rsqrt(sum_c x^2)), and
computes its [2048, 4096] block of the Gram matrix of y with fp32r matmuls.
"""

import sys

sys.path.insert(0, "/opt/trn_rl_repo")

import numpy as np

import concourse.bass as bass
import concourse.mybir as mybir
import concourse.tile as tile
from concourse import bacc
from concourse.bass_utils import run_bass_kernel_spmd
from concourse.vector_clock import ScopedClock, VectorClock

B, C, N = 4, 64, 4096
NCORES = 8
RB = N * B // NCORES  # 2048 output rows per core
MM_N = 512  # moving free dim per matmul (one PSUM bank of fp32)
MM_M = 128  # output partitions per matmul
NJ = N // MM_N  # 8 column chunks
NT = RB // MM_M  # 16 row tiles per core

F32 = mybir.dt.float32
F32R = mybir.dt.float32r


class SplitDrainTileContext(tile.TileContext):
    """Stock TileContext attaches a wait for every pending DMA-queue
    semaphore to a single exit Drain; the walrus build here only allows one
    sync-wait per TPB_CTRL instruction ("Too many sync wait commands").
    Emit one drain per pending logical processor instead."""

    def _drain_and_barrier(self, tick_clock, wait_clock):
        gc = tick_clock.global_clock
        n = len(gc)
        for p in range(n):
            t = gc[p]
            if t <= 0:
                continue
            part = VectorClock([t if q == p else 0 for q in range(n)])
            d = self.nc.sync.drain()
            wait_clock.add_sem_waits(d.ins, ScopedClock({None: part}))

        self.nc.all_engine_barrier()
        assert self.sems is not None
        popped = self.nc._tile_sem_poison_stack.pop()
        assert popped is self._sem_poison
        self.nc.clear_and_free_semaphores(list(self.sems.allocated().values()))
        self.nc.all_engine_barrier()


def _build(use_split_drain=False):
    nc = bacc.Bacc("TRN2", target_bir_lowering=False)
    xf = nc.declare_dram_parameter("xf", [C, N], F32, isOutput=False)
    xr = nc.declare_dram_parameter("xr", [C, RB], F32, isOutput=False)
    out = nc.declare_dram_parameter("out", [RB, N], F32, isOutput=True)
    rnf_d = nc.dram_tensor("rnf_bounce", [1, N], F32)
    rnr_d = nc.dram_tensor("rnr_bounce", [1, RB], F32)

    tc_cls = SplitDrainTileContext if use_split_drain else tile.TileContext
    with tc_cls(nc) as tc:
        with (
            tc.tile_pool(name="persist", bufs=1) as persist,
            tc.tile_pool(name="panels", bufs=3) as panels,
            tc.tile_pool(name="mpsum", bufs=2, space="PSUM") as mpsum,
        ):
            # Load inputs.
            XF = persist.tile([C, N], F32)
            XR = persist.tile([C, RB], F32)
            nc.sync.dma_start(out=XR, in_=xr[:, :])
            nc.sync.dma_start(out=XF, in_=xf[:, :])

            ones_f = persist.tile([C, 1], F32)
            nc.vector.memset(ones_f, 1.0)
            ones = persist.tile([C, 1], F32R)
            nc.vector.tensor_copy(ones, ones_f)

            # ---- Row path (gates the matmuls): normalize this core's row
            # columns: yr = xr * rsqrt(sum_c xr^2).
            SQR = persist.tile([C, RB], F32R)
            nc.vector.tensor_mul(SQR, XR, XR)
            RNR = persist.tile([1, RB], F32)
            nps = mpsum.tile([MM_M, 4 * MM_N], F32, tag="ps")
            for q in range(RB // MM_N):
                js = slice(q * MM_N, (q + 1) * MM_N)
                nc.tensor.matmul(
                    nps[0:1, js], lhsT=ones, rhs=SQR[:, js], start=True, stop=True
                )
            nc.scalar.activation(
                RNR, nps[0:1, :], mybir.ActivationFunctionType.Sqrt
            )
            nc.vector.reciprocal_approx_fast(RNR, RNR)
            BR = persist.tile([C, RB], F32)
            nc.sync.dma_start(out=rnr_d[:, :], in_=RNR)
            nc.sync.dma_start(out=BR, in_=rnr_d[:, :].to_broadcast([C, RB]))
            YR = persist.tile([C, RB], F32R)
            nc.vector.tensor_mul(YR, XR, BR)

            # fp32r-rounded copy of the full x (moving operand). On the
            # scalar engine to keep DVE free.
            XFr = persist.tile([C, N], F32R)
            nc.scalar.copy(out=XFr, in_=XF)

            # ---- Column path (gates only the PSUM->SBUF stage): reciprocal
            # norms of all N columns, broadcast to 128 partitions.
            SQF = persist.tile([C, N], F32R)
            nc.vector.tensor_mul(SQF, XF, XF)
            RNF = persist.tile([1, N], F32)
            R128 = persist.tile([MM_M, N], F32)
            for h in range(2):
                hs = slice(2048 * h, 2048 * (h + 1))
                nps = mpsum.tile([MM_M, 4 * MM_N], F32, tag="ps")
                for q in range(4):
                    j = 4 * h + q
                    js = slice(j * MM_N, (j + 1) * MM_N)
                    qs = slice(q * MM_N, (q + 1) * MM_N)
                    nc.tensor.matmul(
                        nps[0:1, qs], lhsT=ones, rhs=SQF[:, js], start=True, stop=True
                    )
                nc.scalar.activation(
                    RNF[:, hs],
                    nps[0:1, :],
                    mybir.ActivationFunctionType.Sqrt,
                )
                nc.vector.reciprocal_approx_fast(RNF[:, hs], RNF[:, hs])
                nc.sync.dma_start(out=rnf_d[:, hs], in_=RNF[:, hs])
                nc.sync.dma_start(
                    out=R128[:, hs], in_=rnf_d[:, hs].to_broadcast([MM_M, 2048])
                )

            # ---- Main loop: out[i, j] = (yr_i . x_j) * rnf_j.
            # 4 matmuls fill a 4-bank PSUM tile; one DVE multiply folds the
            # column normalization while moving PSUM->SBUF; one contiguous
            # 2 MiB DMA per 128-row panel.
            for t in range(NT):
                panel = panels.tile([MM_M, N], F32)
                ts_ = slice(t * MM_M, (t + 1) * MM_M)
                for h in range(2):
                    ps = mpsum.tile([MM_M, 4 * MM_N], F32, tag="ps")
                    for q in range(4):
                        j = 4 * h + q
                        js = slice(j * MM_N, (j + 1) * MM_N)
                        qs = slice(q * MM_N, (q + 1) * MM_N)
                        nc.tensor.matmul(
                            ps[:, qs],
                            lhsT=YR[:, ts_],
                            rhs=XFr[:, js],
                            start=True,
                            stop=True,
                        )
                    hs = slice(h * 2048, (h + 1) * 2048)
                    nc.vector.tensor_mul(panel[:, hs], ps, R128[:, hs])
                nc.sync.dma_start(out=out[ts_, :], in_=panel)

    nc.compile()
    return nc


def _install_profile_hook():
    """This container's antenv lacks axon_hooks, so run_bass_kernel_spmd's
    trace=True path dies on import. Recreate the module and register the
    ctypes NTFF hook that trn_boot would have installed."""
    import sys as _sys
    import types

    if "antenv.axon_hooks" in _sys.modules:
        return
    import antenv

    mod = types.ModuleType("antenv.axon_hooks")
    mod._hook = None

    def set_axon_ntff_profile_hook(h):
        mod._hook = h

    def get_axon_ntff_profile_hook():
        return mod._hook

    mod.set_axon_ntff_profile_hook = set_axon_ntff_profile_hook
    mod.get_axon_ntff_profile_hook = get_axon_ntff_profile_hook
    _sys.modules["antenv.axon_hooks"] = mod
    antenv.axon_hooks = mod

    from trn_agent_boot.trn_boot import _ntff_profile_via_ctypes

    mod.set_axon_ntff_profile_hook(
        _ntff_profile_via_ctypes("/opt/axon/libaxon_pjrt.so")
    )


_nc = None


def _get_nc():
    global _nc
    if _nc is None:
        _nc = _build()
    return _nc


def _run(x, trace=False, trace_cores=None):
    x = np.asarray(x, dtype=np.float32)
    assert x.shape == (B, C, N), x.shape
    core_ids = list(range(NCORES))
    in_maps = []
    for k in core_ids:
        b, r = divmod(k, 2)
        in_maps.append(
            {
                "xf": np.ascontiguousarray(x[b]),
                "xr": np.ascontiguousarray(x[b][:, r * RB : (r + 1) * RB]),
            }
        )
    if trace:
        _install_profile_hook()
    res = run_bass_kernel_spmd(
        _get_nc(), in_maps, core_ids, trace=trace, trace_cores=trace_cores
    )
    out = np.empty((B, N, N), dtype=np.float32)
    for k in core_ids:
        b, r = divmod(k, 2)
        out[b, r * RB : (r + 1) * RB, :] = res.results[k]["out"]
    return out, res


def kernel(x):
    return _run(x)[0]
